# revision 6
# baseline (speedup 1.0000x reference)
"""Trainium2 Bass kernel for nn_BaseEncoder (ragged entity-pair encoder).

Contract: kernel(**inputs) takes the FULL unsharded inputs (numpy) and
returns the FULL output [B, Q, E, E, R] float32.

Sharding: B*Q = 8 independent (batch, query) pairs -> one per NeuronCore.
Small weights (W_head / W_tail / prototypes-for-that-b) are replicated.

Host-side prep per core (cheap, index/layout only):
  - gather the E*M mention rows of the per-query attention and sum over the
    M=2 mentions (the /2 and /NH scalings cancel in the later row-softmax-
    style normalization, so they are dropped),
  - transpose to At[l, (h,e)] so the device never needs a transpose,
  - entity means ent = mean_m seq[pos] (transposed to entT),
  - prototypes for this b, reshaped/transposed to [2H, R*P].

Device kernel per core (all fp32):
  mul[l, e*32+f] = sum_h At[l,h,e] * At[l,h,f]              (VectorE)
  S[ef]   = sum_l mul[l, ef]                                 (TensorE, ones)
  ctxT[h', ef] = sum_l seq[l, h'] * mul[l, ef]               (TensorE)
  ctxnT = ctxT * (1/S)                                       (VectorE)
  epH[h'', e] = sum_h' W_head[h', h''] entT[h', e]  (and tail)    (TensorE)
  hT[h'', ef] = tanh(sum_h' W_head[768+h', h''] ctxnT[h', ef] + epH[h'', e])
  tT[h'', ef] = tanh(... W_tail ... + epT[h'', f])       (TensorE+VectorE+ScalarE)
  scores[ef, rp] = sum_d candT[d, ef] * protoT[d, rp]        (TensorE)
  out[ef, r] = max_p scores[ef, r*10+p]                      (VectorE)
"""

import numpy as np

B, Q, L, H, E, M, R, P, NH = 2, 4, 1024, 768, 32, 2, 5, 10, 12
NCORES = 8
LT = L // 128          # 8 l-tiles
HT = H // 128          # 6 tiles of 128 along a hidden dim
EF = E * E             # 1024 entity pairs
RP = R * P             # 50 prototype rows

_CACHE = {}


def _build_program():
    import concourse.mybir as mybir
    import concourse.tile as tile
    from concourse import bacc

    f32 = mybir.dt.float32
    nc = bacc.Bacc("TRN2", target_bir_lowering=False, debug=False,
                   num_devices=NCORES)

    at_d = nc.dram_tensor("at", [L, NH * E], f32, kind="ExternalInput").ap()
    seq_d = nc.dram_tensor("seq", [L, H], f32, kind="ExternalInput").ap()
    entT_d = nc.dram_tensor("entT", [H, E], f32, kind="ExternalInput").ap()
    wh_d = nc.dram_tensor("wh", [2 * H, H], f32, kind="ExternalInput").ap()
    wt_d = nc.dram_tensor("wt", [2 * H, H], f32, kind="ExternalInput").ap()
    ptT_d = nc.dram_tensor("ptT", [2 * H, RP], f32, kind="ExternalInput").ap()
    out_d = nc.dram_tensor("out", [EF, R], f32, kind="ExternalOutput").ap()

    with tile.TileContext(nc) as tc:
        _emit(tc, mybir, at_d, seq_d, entT_d, wh_d, wt_d, ptT_d, out_d)

    nc.compile()
    return nc


def _emit(tc, mybir, at_d, seq_d, entT_d, wh_d, wt_d, ptT_d, out_d):
    nc = tc.nc
    f32 = mybir.dt.float32
    Alu = mybir.AluOpType
    Act = mybir.ActivationFunctionType
    Ax = mybir.AxisListType

    import contextlib
    ctx = contextlib.ExitStack()
    with ctx:
        const = ctx.enter_context(tc.tile_pool(name="const", bufs=1))
        big = ctx.enter_context(tc.tile_pool(name="big", bufs=1))
        # 12 slots of [128, EF] shared by the 8 mul tiles (phase 1-2) and the
        # 12 candT tiles (phase 3-4): mul tiles die before most candT tiles
        # are written, so 12 slots suffice for both.
        mulp = ctx.enter_context(tc.tile_pool(name="mulp", bufs=2 * HT))
        tmp = ctx.enter_context(tc.tile_pool(name="tmp", bufs=2))
        # PSUM budget is 8 banks total, statically split:
        #   tag "acc":   3 slots x [128, 2, 512] (2 banks each) = 6 banks
        #   tag "small": 2 slots x 1 bank                       = 2 banks
        psum = ctx.enter_context(tc.tile_pool(name="psum", bufs=1, space="PSUM"))

        # ---------------- input loads ----------------
        at_sb = big.tile([128, LT, NH * E], f32, tag="at_sb")
        nc.sync.dma_start(out=at_sb, in_=at_d.rearrange("(t p) n -> p t n", p=128))
        seq_sb = big.tile([128, LT, H], f32, tag="seq_sb")
        nc.sync.dma_start(out=seq_sb, in_=seq_d.rearrange("(t p) n -> p t n", p=128))
        entT_sb = const.tile([128, HT, E], f32, tag="entT_sb")
        nc.sync.dma_start(out=entT_sb, in_=entT_d.rearrange("(t p) n -> p t n", p=128))
        ptT_sb = const.tile([128, 2 * HT, RP], f32, tag="ptT_sb")
        nc.sync.dma_start(out=ptT_sb, in_=ptT_d.rearrange("(t p) n -> p t n", p=128))
        wh_sb = big.tile([128, 2 * HT, H], f32, tag="wh_sb")
        nc.sync.dma_start(out=wh_sb, in_=wh_d.rearrange("(t p) n -> p t n", p=128))
        wt_sb = big.tile([128, 2 * HT, H], f32, tag="wt_sb")
        nc.sync.dma_start(out=wt_sb, in_=wt_d.rearrange("(t p) n -> p t n", p=128))

        ones_col = const.tile([128, 1], f32, tag="ones_col")
        nc.vector.memset(ones_col, 1.0)
        ones_row = const.tile([1, 128], f32, tag="ones_row")
        nc.vector.memset(ones_row, 1.0)

        # ---------------- stage 1: mul (VectorE) + S/ctx-groupA (TensorE) ----
        mul_t = []
        s_ps = [psum.tile([1, 512], f32, tag="small", bufs=2, name=f"s_ps{c}")
                for c in range(2)]
        ctxA_ps = [psum.tile([128, 2, 512], f32, tag="acc", bufs=3,
                             name=f"ctxA{ht}") for ht in range(3)]

        for lt in range(LT):
            at3 = at_sb[:, lt, :].rearrange("p (h e) -> p h e", h=NH)
            mt = mulp.tile([128, EF], f32, tag="mul_t")
            mul_t.append(mt)
            m3 = mt.rearrange("p (e f) -> p e f", e=E)
            for h in range(NH):
                a = at3[:, h, :]
                a_e = a[:, :, None].broadcast_to([128, E, E])
                a_f = a[:, None, :].broadcast_to([128, E, E])
                if h == 0:
                    nc.vector.tensor_mul(m3, a_e, a_f)
                else:
                    t = tmp.tile([128, E, E], f32, tag="scratch")
                    nc.vector.tensor_mul(t, a_e, a_f)
                    nc.vector.tensor_add(m3, m3, t)

            first, last = (lt == 0), (lt == LT - 1)
            for c in range(2):
                nc.tensor.matmul(s_ps[c], ones_col,
                                 mt[:, c * 512:(c + 1) * 512],
                                 start=first, stop=last)
            for ht in range(3):
                for c in range(2):
                    nc.tensor.matmul(
                        ctxA_ps[ht][:, c, :],
                        seq_sb[:, lt, ht * 128:(ht + 1) * 128],
                        mt[:, c * 512:(c + 1) * 512],
                        start=first, stop=last)

        # ---------------- 1/S, broadcast to 128 partitions ----------------
        rec1 = const.tile([1, EF], f32, tag="rec1")
        for c in range(2):
            nc.vector.tensor_copy(rec1[:, c * 512:(c + 1) * 512], s_ps[c])
        nc.vector.reciprocal(rec1, rec1)
        # replicate 1/S across all 128 partitions: ones[1,128].T @ rec1-chunk
        recS_sb = big.tile([128, EF], f32, tag="recS_sb")
        for c in range(2):
            rb = psum.tile([128, 512], f32, tag="small", bufs=2, name="recB")
            nc.tensor.matmul(rb, ones_row, rec1[:, c * 512:(c + 1) * 512],
                             start=True, stop=True)
            nc.vector.tensor_copy(recS_sb[:, c * 512:(c + 1) * 512], rb)

        # ---------------- ctx: normalize group A, run group B ----------------
        ctxnT_sb = big.tile([128, HT, EF], f32, tag="ctxnT_sb")
        for ht in range(3):
            nc.vector.tensor_mul(ctxnT_sb[:, ht, :],
                                 ctxA_ps[ht].rearrange("p a b -> p (a b)"),
                                 recS_sb)
        for ht in range(3, HT):
            ps = psum.tile([128, 2, 512], f32, tag="acc", bufs=3, name="ctxB")
            for lt in range(LT):
                for c in range(2):
                    nc.tensor.matmul(
                        ps[:, c, :],
                        seq_sb[:, lt, ht * 128:(ht + 1) * 128],
                        mul_t[lt][:, c * 512:(c + 1) * 512],
                        start=(lt == 0), stop=(lt == LT - 1))
            nc.vector.tensor_mul(ctxnT_sb[:, ht, :],
                                 ps.rearrange("p a b -> p (a b)"), recS_sb)

        # ---------------- entity projections (ent @ W[:H]) ----------------
        ep_sb = []
        for w, wsb in ((0, wh_sb), (1, wt_sb)):
            ep = const.tile([128, HT, E], f32, tag=f"ep{w}")
            ep_sb.append(ep)
            for ht2 in range(HT):
                ps = psum.tile([128, E], f32, tag="small", bufs=2, name="ep_ps")
                for kt in range(HT):
                    nc.tensor.matmul(ps, wsb[:, kt, ht2 * 128:(ht2 + 1) * 128],
                                     entT_sb[:, kt, :],
                                     start=(kt == 0), stop=(kt == HT - 1))
                nc.scalar.copy(ep[:, ht2, :], ps)

        # ---------------- pair projections + bias + tanh ----------------
        candT_t = []
        for w, wsb in ((0, wh_sb), (1, wt_sb)):
            for ht2 in range(HT):
                ps = psum.tile([128, 2, 512], f32, tag="acc", bufs=3, name="proj_ps")
                for kt in range(HT):
                    for c in range(2):
                        nc.tensor.matmul(
                            ps[:, c, :],
                            wsb[:, HT + kt, ht2 * 128:(ht2 + 1) * 128],
                            ctxnT_sb[:, kt, c * 512:(c + 1) * 512],
                            start=(kt == 0), stop=(kt == HT - 1))
                if w == 0:
                    bias = ep_sb[0][:, ht2, :, None].broadcast_to([128, E, E])
                else:
                    bias = ep_sb[1][:, ht2, None, :].broadcast_to([128, E, E])
                pre = tmp.tile([128, EF], f32, tag="scratch")
                nc.vector.tensor_add(pre.rearrange("p (e f) -> p e f", e=E),
                                     ps.rearrange("p a b -> p (a b)")
                                       .rearrange("p (e f) -> p e f", e=E),
                                     bias)
                cd = mulp.tile([128, EF], f32, tag="mul_t", name="candT")
                candT_t.append(cd)
                nc.scalar.activation(cd, pre, Act.Tanh)

        # ---------------- scores + max over prototypes ----------------
        out_sb = const.tile([128, LT, R], f32, tag="out_sb")
        for et in range(LT):
            ps = psum.tile([128, RP], f32, tag="small", bufs=2, name="sc_ps")
            for kt in range(2 * HT):
                nc.tensor.matmul(ps, candT_t[kt][:, et * 128:(et + 1) * 128],
                                 ptT_sb[:, kt, :],
                                 start=(kt == 0), stop=(kt == 2 * HT - 1))
            nc.vector.tensor_reduce(
                out=out_sb[:, et, :],
                in_=ps.rearrange("p (r q) -> p r q", r=R),
                axis=Ax.X, op=Alu.max)
        nc.sync.dma_start(out=out_d.rearrange("(t p) r -> p t r", p=128),
                          in_=out_sb)


def _host_prep(sequence_output, attention, W_head, W_tail, prototypes,
               mention_pos):
    """Build the per-core input maps (numpy only)."""
    seq = np.ascontiguousarray(sequence_output, dtype=np.float32)
    att = np.asarray(attention, dtype=np.float32)
    wh = np.ascontiguousarray(W_head, dtype=np.float32)
    wt = np.ascontiguousarray(W_tail, dtype=np.float32)
    pro = np.asarray(prototypes, dtype=np.float32)
    pos = np.asarray(mention_pos)

    in_maps = []
    for c in range(NCORES):
        b, q = divmod(c, Q)
        p_bq = pos[b, q]                       # [E, M]
        # attention gather + mention-sum: [NH, E, L] (scale dropped)
        g = att[b, q][:, p_bq, :]              # [NH, E, M, L]
        asum = g[:, :, 0, :] + g[:, :, 1, :]   # [NH, E, L]
        at = np.ascontiguousarray(
            asum.reshape(NH * E, L).T)         # [L, NH*E], At[l, h*E+e]
        # entity means: [E, H] -> entT [H, E]
        ment = seq[b, q][p_bq]                 # [E, M, H]
        ent = (ment[:, 0, :] + ment[:, 1, :]) * np.float32(0.5)
        entT = np.ascontiguousarray(ent.T)
        ptT = np.ascontiguousarray(
            pro[b].reshape(RP, 2 * H).T)       # [2H, RP]
        in_maps.append({
            "at": at,
            "seq": seq[b, q],
            "entT": entT,
            "wh": wh,
            "wt": wt,
            "ptT": ptT,
        })
    return in_maps


def kernel(sequence_output, attention, W_head, W_tail, prototypes,
           mention_pos):
    from concourse.bass_utils import run_bass_kernel_spmd

    if "nc" not in _CACHE:
        _CACHE["nc"] = _build_program()
    nc = _CACHE["nc"]

    in_maps = _host_prep(sequence_output, attention, W_head, W_tail,
                         prototypes, mention_pos)
    res = run_bass_kernel_spmd(nc, in_maps, core_ids=list(range(NCORES)))

    out = np.empty((B, Q, E, E, R), dtype=np.float32)
    for c in range(NCORES):
        b, q = divmod(c, Q)
        out[b, q] = res.results[c]["out"].reshape(E, E, R)
    return out


# revision 10
# speedup vs baseline: 1.2979x; 1.2979x over previous
"""Trainium2 Bass kernel for nn_BaseEncoder (ragged entity-pair encoder).

Contract: kernel(**inputs) takes the FULL unsharded inputs (numpy) and
returns the FULL output [B, Q, E, E, R] float32.

Sharding: B*Q = 8 independent (batch, query) pairs -> one per NeuronCore.
Small weights (W_head / W_tail / prototypes-for-that-b) are replicated.

Host-side prep per core (cheap, index/layout only):
  - gather the E*M mention rows of the per-query attention and sum over the
    M=2 mentions (the /2 and /NH scalings cancel in the later row-softmax-
    style normalization, so they are dropped),
  - transpose to At[l, (h,e)] so the device never needs a transpose,
  - entity means ent = mean_m seq[pos] (transposed to entT),
  - prototypes for this b, reshaped/transposed to [2H, R*P].

Device kernel per core (all fp32):
  mul[l, e*32+f] = sum_h At[l,h,e] * At[l,h,f]              (VectorE)
  S[ef]   = sum_l mul[l, ef]                                 (TensorE, ones)
  ctxT[h', ef] = sum_l seq[l, h'] * mul[l, ef]               (TensorE)
  ctxnT = ctxT * (1/S)                                       (VectorE)
  epH[h'', e] = sum_h' W_head[h', h''] entT[h', e]  (and tail)    (TensorE)
  hT[h'', ef] = tanh(sum_h' W_head[768+h', h''] ctxnT[h', ef] + epH[h'', e])
  tT[h'', ef] = tanh(... W_tail ... + epT[h'', f])       (TensorE+VectorE+ScalarE)
  scores[ef, rp] = sum_d candT[d, ef] * protoT[d, rp]        (TensorE)
  out[ef, r] = max_p scores[ef, r*10+p]                      (VectorE)
"""

import numpy as np

B, Q, L, H, E, M, R, P, NH = 2, 4, 1024, 768, 32, 2, 5, 10, 12
NCORES = 8
LT = L // 128          # 8 l-tiles
HT = H // 128          # 6 tiles of 128 along a hidden dim
EF = E * E             # 1024 entity pairs
RP = R * P             # 50 prototype rows

_CACHE = {}


def _build_program():
    import concourse.mybir as mybir
    import concourse.tile as tile
    from concourse import bacc

    f32 = mybir.dt.float32
    nc = bacc.Bacc("TRN2", target_bir_lowering=False, debug=False,
                   num_devices=NCORES)

    at_d = nc.dram_tensor("at", [L, NH * E], f32, kind="ExternalInput").ap()
    seq_d = nc.dram_tensor("seq", [L, H], mybir.dt.float32r, kind="ExternalInput").ap()
    entT_d = nc.dram_tensor("entT", [H, E], f32, kind="ExternalInput").ap()
    wh_d = nc.dram_tensor("wh", [2 * H, H], mybir.dt.float32r, kind="ExternalInput").ap()
    wt_d = nc.dram_tensor("wt", [2 * H, H], mybir.dt.float32r, kind="ExternalInput").ap()
    ptT_d = nc.dram_tensor("ptT", [2 * H, RP], f32, kind="ExternalInput").ap()
    out_d = nc.dram_tensor("out", [EF, R], f32, kind="ExternalOutput").ap()

    with tile.TileContext(nc) as tc:
        _emit(tc, mybir, at_d, seq_d, entT_d, wh_d, wt_d, ptT_d, out_d)

    nc.compile()
    return nc


USE_F32R = True


def _emit(tc, mybir, at_d, seq_d, entT_d, wh_d, wt_d, ptT_d, out_d):
    nc = tc.nc
    f32 = mybir.dt.float32
    f32r = mybir.dt.float32r

    Alu = mybir.AluOpType
    Act = mybir.ActivationFunctionType
    Ax = mybir.AxisListType

    import contextlib
    ctx = contextlib.ExitStack()
    with ctx:
        const = ctx.enter_context(tc.tile_pool(name="const", bufs=1))
        big = ctx.enter_context(tc.tile_pool(name="big", bufs=1))
        # 12 slots of [128, EF] shared by the 8 mul tiles (phase 1-2) and the
        # 12 candT tiles (phase 3-4): mul tiles die before most candT tiles
        # are written, so 12 slots suffice for both.
        mulp = ctx.enter_context(tc.tile_pool(name="mulp", bufs=2 * HT))
        tmp = ctx.enter_context(tc.tile_pool(name="tmp", bufs=2))
        # PSUM budget is 8 banks total, statically split:
        #   tag "acc":   3 slots x [128, 2, 512] (2 banks each) = 6 banks
        #   tag "small": 2 slots x 1 bank                       = 2 banks
        psum = ctx.enter_context(tc.tile_pool(name="psum", bufs=1, space="PSUM"))

        # ---------------- input loads ----------------
        at_sb = big.tile([128, LT, NH * E], f32, tag="at_sb")
        nc.sync.dma_start(out=at_sb, in_=at_d.rearrange("(t p) n -> p t n", p=128))
        seq_sb = big.tile([128, LT, H], f32r, tag="seq_sb")
        nc.sync.dma_start(out=seq_sb, in_=seq_d.rearrange("(t p) n -> p t n", p=128))
        entT_sb = const.tile([128, HT, E], f32, tag="entT_sb")
        nc.sync.dma_start(out=entT_sb, in_=entT_d.rearrange("(t p) n -> p t n", p=128))
        ptT_sb = const.tile([128, 2 * HT, RP], f32, tag="ptT_sb")
        nc.sync.dma_start(out=ptT_sb, in_=ptT_d.rearrange("(t p) n -> p t n", p=128))
        wh_sb = big.tile([128, 2 * HT, H], f32r, tag="wh_sb")
        nc.sync.dma_start(out=wh_sb, in_=wh_d.rearrange("(t p) n -> p t n", p=128))
        wt_sb = big.tile([128, 2 * HT, H], f32r, tag="wt_sb")
        nc.sync.dma_start(out=wt_sb, in_=wt_d.rearrange("(t p) n -> p t n", p=128))

        ones_col = const.tile([128, 1], f32, tag="ones_col")
        nc.vector.memset(ones_col, 1.0)
        ones_row = const.tile([1, 128], f32, tag="ones_row")
        nc.vector.memset(ones_row, 1.0)

        # ---------------- stage 1: mul (VectorE) + S/ctx-groupA (TensorE) ----
        mul_t = []
        s_ps = [psum.tile([1, 512], f32, tag="small", bufs=2, name=f"s_ps{c}")
                for c in range(2)]
        ctxA_ps = [psum.tile([128, 2, 512], f32, tag="acc", bufs=3,
                             name=f"ctxA{ht}") for ht in range(3)]

        for lt in range(LT):
            at3 = at_sb[:, lt, :].rearrange("p (h e) -> p h e", h=NH)
            mt = mulp.tile([128, EF], f32r, tag="mul_t")
            mul_t.append(mt)
            m3 = mt.rearrange("p (e f) -> p e f", e=E)
            for h in range(NH):
                a = at3[:, h, :]
                a_e = a[:, :, None].broadcast_to([128, E, E])
                a_f = a[:, None, :].broadcast_to([128, E, E])
                if h == 0:
                    nc.vector.tensor_mul(m3, a_e, a_f)
                else:
                    t = tmp.tile([128, E, E], f32, tag="scratch")
                    nc.vector.tensor_mul(t, a_e, a_f)
                    nc.vector.tensor_add(m3, m3, t)

            first, last = (lt == 0), (lt == LT - 1)
            for c in range(2):
                nc.tensor.matmul(s_ps[c], ones_col.bitcast(f32r),
                                 mt[:, c * 512:(c + 1) * 512],
                                 start=first, stop=last)
            for ht in range(3):
                for c in range(2):
                    nc.tensor.matmul(
                        ctxA_ps[ht][:, c, :],
                        seq_sb[:, lt, ht * 128:(ht + 1) * 128],
                        mt[:, c * 512:(c + 1) * 512],
                        start=first, stop=last)

        # ---------------- 1/S, broadcast to 128 partitions ----------------
        rec1 = const.tile([1, EF], f32, tag="rec1")
        for c in range(2):
            nc.vector.tensor_copy(rec1[:, c * 512:(c + 1) * 512], s_ps[c])
        nc.vector.reciprocal(rec1, rec1)
        # replicate 1/S across all 128 partitions: ones[1,128].T @ rec1-chunk
        recS_sb = big.tile([128, EF], f32, tag="recS_sb")
        for c in range(2):
            rb = psum.tile([128, 512], f32, tag="small", bufs=2, name="recB")
            nc.tensor.matmul(rb, ones_row, rec1[:, c * 512:(c + 1) * 512],
                             start=True, stop=True)
            nc.vector.tensor_copy(recS_sb[:, c * 512:(c + 1) * 512], rb)

        # ---------------- ctx: normalize group A, run group B ----------------
        ctxnT_sb = big.tile([128, HT, EF], f32r, tag="ctxnT_sb")
        for ht in range(3):
            nc.vector.tensor_mul(ctxnT_sb[:, ht, :],
                                 ctxA_ps[ht].rearrange("p a b -> p (a b)"),
                                 recS_sb)
        for ht in range(3, HT):
            ps = psum.tile([128, 2, 512], f32, tag="acc", bufs=3, name="ctxB")
            for lt in range(LT):
                for c in range(2):
                    nc.tensor.matmul(
                        ps[:, c, :],
                        seq_sb[:, lt, ht * 128:(ht + 1) * 128],
                        mul_t[lt][:, c * 512:(c + 1) * 512],
                        start=(lt == 0), stop=(lt == LT - 1))
            nc.vector.tensor_mul(ctxnT_sb[:, ht, :],
                                 ps.rearrange("p a b -> p (a b)"), recS_sb)

        # ---------------- entity projections (ent @ W[:H]) ----------------
        ep_sb = []
        for w, wsb in ((0, wh_sb), (1, wt_sb)):
            ep = const.tile([128, HT, E], f32, tag=f"ep{w}")
            ep_sb.append(ep)
            for ht2 in range(HT):
                ps = psum.tile([128, E], f32, tag="small", bufs=2, name="ep_ps")
                for kt in range(HT):
                    nc.tensor.matmul(
                        ps, wsb[:, kt, ht2 * 128:(ht2 + 1) * 128].bitcast(f32),
                        entT_sb[:, kt, :],
                        start=(kt == 0), stop=(kt == HT - 1))
                nc.scalar.copy(ep[:, ht2, :], ps)

        # ---------------- pair projections + bias + tanh ----------------
        candT_t = []
        for w, wsb in ((0, wh_sb), (1, wt_sb)):
            for ht2 in range(HT):
                ps = psum.tile([128, 2, 512], f32, tag="acc", bufs=3, name="proj_ps")
                for kt in range(HT):
                    for c in range(2):
                        nc.tensor.matmul(
                            ps[:, c, :],
                            wsb[:, HT + kt, ht2 * 128:(ht2 + 1) * 128],
                            ctxnT_sb[:, kt, c * 512:(c + 1) * 512],
                            start=(kt == 0), stop=(kt == HT - 1))
                if w == 0:
                    bias = ep_sb[0][:, ht2, :, None].broadcast_to([128, E, E])
                else:
                    bias = ep_sb[1][:, ht2, None, :].broadcast_to([128, E, E])
                pre = tmp.tile([128, EF], f32, tag="scratch")
                nc.vector.tensor_add(pre.rearrange("p (e f) -> p e f", e=E),
                                     ps.rearrange("p a b -> p (a b)")
                                       .rearrange("p (e f) -> p e f", e=E),
                                     bias)
                cd = mulp.tile([128, EF], f32, tag="mul_t", name="candT")
                candT_t.append(cd)
                nc.scalar.activation(cd, pre, Act.Tanh)

        # ---------------- scores + max over prototypes ----------------
        out_sb = const.tile([128, LT, R], f32, tag="out_sb")
        for et in range(LT):
            ps = psum.tile([128, RP], f32, tag="small", bufs=2, name="sc_ps")
            for kt in range(2 * HT):
                nc.tensor.matmul(ps, candT_t[kt][:, et * 128:(et + 1) * 128],
                                 ptT_sb[:, kt, :],
                                 start=(kt == 0), stop=(kt == 2 * HT - 1))
            nc.vector.tensor_reduce(
                out=out_sb[:, et, :],
                in_=ps.rearrange("p (r q) -> p r q", r=R),
                axis=Ax.X, op=Alu.max)
        nc.sync.dma_start(out=out_d.rearrange("(t p) r -> p t r", p=128),
                          in_=out_sb)


def _host_prep(sequence_output, attention, W_head, W_tail, prototypes,
               mention_pos):
    """Build the per-core input maps (numpy only)."""
    seq = np.ascontiguousarray(sequence_output, dtype=np.float32)
    att = np.asarray(attention, dtype=np.float32)
    wh = np.ascontiguousarray(W_head, dtype=np.float32)
    wt = np.ascontiguousarray(W_tail, dtype=np.float32)
    pro = np.asarray(prototypes, dtype=np.float32)
    pos = np.asarray(mention_pos)

    in_maps = []
    for c in range(NCORES):
        b, q = divmod(c, Q)
        p_bq = pos[b, q]                       # [E, M]
        # attention gather + mention-sum: [NH, E, L] (scale dropped)
        g = att[b, q][:, p_bq, :]              # [NH, E, M, L]
        asum = g[:, :, 0, :] + g[:, :, 1, :]   # [NH, E, L]
        at = np.ascontiguousarray(
            asum.reshape(NH * E, L).T)         # [L, NH*E], At[l, h*E+e]
        # entity means: [E, H] -> entT [H, E]
        ment = seq[b, q][p_bq]                 # [E, M, H]
        ent = (ment[:, 0, :] + ment[:, 1, :]) * np.float32(0.5)
        entT = np.ascontiguousarray(ent.T)
        ptT = np.ascontiguousarray(
            pro[b].reshape(RP, 2 * H).T)       # [2H, RP]
        in_maps.append({
            "at": at,
            "seq": seq[b, q],
            "entT": entT,
            "wh": wh,
            "wt": wt,
            "ptT": ptT,
        })
    return in_maps


def kernel(sequence_output, attention, W_head, W_tail, prototypes,
           mention_pos):
    from concourse.bass_utils import run_bass_kernel_spmd

    if "nc" not in _CACHE:
        _CACHE["nc"] = _build_program()
    nc = _CACHE["nc"]

    in_maps = _host_prep(sequence_output, attention, W_head, W_tail,
                         prototypes, mention_pos)
    res = run_bass_kernel_spmd(nc, in_maps, core_ids=list(range(NCORES)))

    out = np.empty((B, Q, E, E, R), dtype=np.float32)
    for c in range(NCORES):
        b, q = divmod(c, Q)
        out[b, q] = res.results[c]["out"].reshape(E, E, R)
    return out
